# revision 2
# baseline (speedup 1.0000x reference)
"""Trainium2 Bass kernel for nn_CelltypeScaleLayer (segment gather + scale + transpose).

Reference computation:
    z = x[idx.reshape(-1)] * repeat(weight, M)[:, None]   # (NJ, NCELL)
    out = z.T.reshape(-1)                                 # (NCELL * NJ,)

Sharding: data-parallel over the NJ (gathered-row) axis. Core m owns output
columns j in [m*JPC, (m+1)*JPC) of the (NCELL, NJ) output, i.e. a contiguous
slab of the flattened output. idx/weight shards are tiny and pre-laid-out on
the host (int16 wrapped index layout for SWDGE dma_gather, and a per-128-j-group
weight table).

Precision: x is converted to bf16 on the host and gathered as 1KB rows. This
halves the gather's DMA byte traffic (the dominant read) while keeping max
relative error ~2^-8, well under the 2e-2 gate. The fp32 output is recovered
for free in the PE stage: instead of a transpose against the identity, each
128x128 j-block is matmul'd against diag(w) in bf16 with fp32 PSUM output, so
the per-j weight scale, the transpose, and the bf16->fp32 upconvert all happen
in one PE pass. The scalar engine only builds the tiny 128x128 diag tiles (one
per 128 j's) from a host-provided per-group weight table.

Per-core pipeline, per chunk of CHUNK j positions:
  1. SWDGE dma_gather: rows x_bf16[idx[j], :] (1KB each) from HBM into SBUF,
     slot layout gb[j%128, j//128, :].
  2. Per 128-j group g: scalar engine builds wd[:, g, :] = diag(w_group) in
     bf16 from the fp32 identity (activation Copy with per-partition scale).
  3. Per 128-column block q: PE matmul ps[:, q*128:(q+1)*128] =
     gb[:, g, q-block].T @ diag(w) -> PSUM fp32 (scaled transpose).
  4. DVE copy PSUM -> output staging buffer (partition = cell column).
  5. HWDGE DMA staging -> HBM output rows (contiguous along j).

DMA byte budget per core: 12.8MB gather read (bf16) + 25.6MB write (fp32)
= 38.4MB @ ~360GB/s per-core DMA = ~107us (vs 51.2MB = ~143us for the fp32
baseline). All compute engines are well below that bound.
"""

import numpy as np
import ml_dtypes

import concourse.bacc as bacc
import concourse.tile as tile
import concourse.mybir as mybir
from concourse import masks
from concourse.bass_utils import run_bass_kernel_spmd

F32 = mybir.dt.float32
BF16 = mybir.dt.bfloat16
I16 = mybir.dt.int16

# Problem shape (hardcoded per the harness contract).
NF = 20000        # x rows (features)
NCELL = 512       # x cols (cells) == output rows
NCT = 50          # celltypes
M = 2000          # rows gathered per celltype
NJ = NCT * M      # 100000 gathered rows == output cols

NCORES = 8
JPC = NJ // NCORES          # 12500 output columns per core
CHUNK = 896                 # gather indices per dma_gather call (7 groups of 128).
                            # >=1280 per SWDGE gather crashes the device
                            # (descriptor-ring capacity is 1024 entries).
GPC = CHUNK // 128          # groups per chunk
NCHUNK = -(-JPC // CHUNK)   # 14
NIDX = NCHUNK * CHUNK       # 12544 (tail padded with -1, skipped by the DMA)
NQ = NCELL // 128           # 4 column blocks

_cached = None


def _build(repeats=1, ncores=NCORES):
    """Build + compile the SPMD program. `repeats` re-runs the whole pipeline
    that many times inside one NEFF (used only for timing measurements)."""
    nc = bacc.Bacc("TRN2", target_bir_lowering=False, debug=False,
                   num_devices=ncores)
    x = nc.dram_tensor("x", [NF, NCELL], BF16, kind="ExternalInput")
    idxs = nc.dram_tensor("idxs", [128, NIDX // 16], I16, kind="ExternalInput")
    wtbl = nc.dram_tensor("wtbl", [128, NIDX // 128], F32, kind="ExternalInput")
    out = nc.dram_tensor("out", [NCELL, JPC], F32, kind="ExternalOutput")

    with tile.TileContext(nc) as tc:
        with tc.tile_pool(name="const", bufs=1) as cpool:
            ident = cpool.tile([128, 128], F32)
            masks.make_identity(nc, ident[:])
            idx_sb = cpool.tile([128, NIDX // 16], I16)
            nc.sync.dma_start(idx_sb[:], idxs.ap())
            wtbl_sb = cpool.tile([128, NIDX // 128], F32)
            nc.sync.dma_start(wtbl_sb[:], wtbl.ap())

            with (
                tc.tile_pool(name="gpool", bufs=4) as gpool,
                tc.tile_pool(name="wpool", bufs=3) as wpool,
                tc.tile_pool(name="opool", bufs=3) as opool,
                tc.tile_pool(name="pspool", bufs=8, space="PSUM") as pspool,
            ):
                for _ in range(repeats):
                    for k in range(NCHUNK):
                        nvalid = min(JPC - k * CHUNK, CHUNK)
                        gb = gpool.tile([128, GPC, NCELL], BF16, tag="gb")
                        if nvalid < CHUNK:
                            # zero the partial group so padded tail slots hold
                            # no garbage (those columns are never DMA'd out,
                            # but they do flow through the PE matmul)
                            nc.vector.memset(gb[:, nvalid // 128, :], 0.0)
                        nc.gpsimd.dma_gather(
                            gb[:],
                            x.ap(),
                            idx_sb[:, k * (CHUNK // 16):(k + 1) * (CHUNK // 16)],
                            CHUNK,
                            nvalid,
                            NCELL,
                        )
                        # per-group diag(w) tiles in bf16, built on the scalar
                        # engine from the fp32 identity (dtype convert is free)
                        wd = wpool.tile([128, GPC, 128], BF16, tag="wd")
                        for g in range(GPC):
                            gcol = k * GPC + g
                            nc.scalar.activation(
                                wd[:, g, :], ident[:],
                                mybir.ActivationFunctionType.Copy,
                                scale=wtbl_sb[:, gcol:gcol + 1],
                            )
                        ob = opool.tile([128, NQ, CHUNK], F32, tag="ob")
                        for g in range(GPC):
                            ps = pspool.tile([128, 512], F32, tag="ps")
                            for q in range(NQ):
                                # ps[c, j'] = sum_j gb[j, c] * diag(w)[j, j']
                                #           = gb[j', c] * w[j']
                                # i.e. scaled transpose with fp32 upconvert
                                nc.tensor.matmul(
                                    ps[:, q * 128:(q + 1) * 128],
                                    gb[:, g, q * 128:(q + 1) * 128],
                                    wd[:, g, :],
                                )
                            ps_v = ps[:, :].rearrange("p (q j) -> p q j", q=NQ)
                            nc.vector.tensor_copy(
                                ob[:, :, g * 128:(g + 1) * 128], ps_v)
                        for q in range(NQ):
                            nc.sync.dma_start(
                                out.ap()[q * 128:(q + 1) * 128,
                                         k * CHUNK:k * CHUNK + nvalid],
                                ob[:, q, :nvalid],
                            )
    nc.compile()
    return nc


def _host_prep(x, weight, idx, ncores=NCORES):
    x_bf = np.ascontiguousarray(
        np.asarray(x, dtype=np.float32).astype(ml_dtypes.bfloat16))
    weight = np.asarray(weight, dtype=np.float32)
    idx_flat = np.asarray(idx).reshape(-1).astype(np.int64)
    w_exp = np.repeat(weight, M).astype(np.float32)  # (NJ,) per-j weight

    in_maps = []
    for m in range(ncores):
        j0 = m * JPC
        padded = np.full((NIDX,), -1, dtype=np.int64)
        padded[:JPC] = idx_flat[j0:j0 + JPC]
        # dma_gather index layout: index i lives at partition i%16, free i//16,
        # replicated across the 8 Q7 core groups.
        wrapped16 = padded.reshape(NIDX // 16, 16).T.astype(np.int16)
        wrapped = np.ascontiguousarray(np.tile(wrapped16, (8, 1)))  # (128, NIDX//16)

        # per-128-j-group weight table: wtbl[p, grp] = w for j = grp*128 + p
        wpad = np.ones((NIDX,), dtype=np.float32)
        wpad[:JPC] = w_exp[j0:j0 + JPC]
        wtbl = np.ascontiguousarray(wpad.reshape(NIDX // 128, 128).T)

        in_maps.append({"x": x_bf, "idxs": wrapped, "wtbl": wtbl})
    return in_maps


def _run(inputs):
    global _cached
    if _cached is None:
        _cached = _build()
    nc = _cached
    in_maps = _host_prep(inputs["x"], inputs["weight"], inputs["idx"])
    res = run_bass_kernel_spmd(nc, in_maps, list(range(NCORES)))
    parts = [res.results[m]["out"] for m in range(NCORES)]
    full = np.concatenate(parts, axis=1)  # (NCELL, NJ)
    return np.ascontiguousarray(full).reshape(-1), res


def kernel(**inputs) -> np.ndarray:
    out, _ = _run(inputs)
    return out
